# revision 1
# baseline (speedup 1.0000x reference)
"""CascadeGDCN (3-hop graph diffusion convolution) on 8 Trainium2 NeuronCores.

Strategy (matches the sharding hint):
  - Destination nodes sharded across the 8 cores (12544 rows each, padded to
    100352 total).  Edges partitioned by destination core.
  - The full feature matrix X is replicated in every core's DRAM; after each
    hop an AllGather rebuilds it from the per-core output shards.
  - Per SpMM: per-core edges are bucketed into cells (128-dest "group" x
    25088-row source "chunk"; the chunk split keeps dma_gather's int16 source
    indices in range).  Messages are fetched with dma_gather (256B rows), the
    segment reduction runs on the TensorEngine as S^T @ M where S is a
    [128 edges x 128 dests] one-hot-times-val matrix built on-chip by two
    batched VectorEngine ops (iota compare + multiply).  PSUM accumulates the
    20 tiles (4 chunks x 5 slots) of each dest group.
  - Final stage: sum_term^T via PE transpose, z = Theta^T @ st_fm on PE,
    sigmoid on ScalarE, + H on VectorE, output written feature-major and
    transposed back on the host.
"""

import numpy as np

D = 64
NCORES = 8
NUM_HOPS = 3
N_NODES = 100000
SHARD = 12544            # dest rows per core (98 groups of 128)
NODES_PAD = SHARD * NCORES   # 100352
NCHUNKS = 4
CHUNK = NODES_PAD // NCHUNKS  # 25088 (< 32768 so chunk-local idx fits int16)
GROUPS = SHARD // 128    # 98
GPB = 4                  # dest groups per block (per gather call)
MIN_CAP = 5              # min slots (128-edge tiles) per (group, chunk) cell
SKIP_FINAL = False       # dev bisect flag
SKIP_SPMM = False        # dev bisect flag


def _softmax(x):
    e = np.exp(x - x.max())
    return e / e.sum()


def _blocks(groups, gpb):
    out = []
    g = 0
    while g < groups:
        out.append((g, min(gpb, groups - g)))
        g += gpb
    return out


def _layout(groups, gpb, cap):
    """Edge-stream layout: [block][chunk][group_in_block][cap*128]."""
    cap_e = cap * 128
    blocks = _blocks(groups, gpb)
    block_base = []
    base = 0
    for _, gc in blocks:
        block_base.append(base)
        base += NCHUNKS * gc * cap_e
    return blocks, block_base, base  # base == total edge slots


def _prep_direction(dest, src, val, shard, groups, gpb, cap, chunk):
    """Build per-core gather/S tables for one SpMM direction.

    Returns list (per core) of dicts with idx/denc/val device tables.
    """
    cap_e = cap * 128
    blocks, block_base, tot = _layout(groups, gpb, cap)
    ncells = groups * NCHUNKS

    # cell base offset for (g, c)
    cell_base = np.empty(ncells, dtype=np.int64)
    for bi, (g0, gc) in enumerate(blocks):
        for gl in range(gc):
            for c in range(NCHUNKS):
                cell_base[(g0 + gl) * NCHUNKS + c] = (
                    block_base[bi] + c * gc * cap_e + gl * cap_e
                )

    core = dest // shard
    out = []
    for m in range(NCORES):
        sel = core == m
        d_loc = (dest[sel] - m * shard).astype(np.int64)
        s = src[sel].astype(np.int64)
        v = val[sel].astype(np.float32)
        g = d_loc >> 7
        c = s // chunk
        cell = g * NCHUNKS + c
        order = np.argsort(cell, kind="stable")
        cell_s = cell[order]
        counts = np.bincount(cell_s, minlength=ncells)
        if counts.max() > cap_e:
            raise OverflowError(int(np.ceil(counts.max() / 128)))
        starts = np.zeros(ncells, dtype=np.int64)
        starts[1:] = np.cumsum(counts)[:-1]
        rank = np.arange(cell_s.size) - starts[cell_s]
        pos = cell_base[cell_s] + rank

        idx_st = np.zeros(tot, dtype=np.int16)
        denc_st = np.full(tot, -1.0, dtype=np.float32)
        val_st = np.zeros(tot, dtype=np.float32)
        idx_st[pos] = (s[order] - c[order] * chunk).astype(np.int16)
        denc_st[pos] = (d_loc[order] & 127).astype(np.float32)
        val_st[pos] = v[order]

        # the 16-row wrapped block must be replicated into all 8 Q7-core
        # stripes (hardware reads its own 16-partition stripe)
        idx_tbl = np.tile(np.ascontiguousarray(idx_st.reshape(-1, 16).T),
                          (8, 1))
        denc_tbl = np.ascontiguousarray(denc_st.reshape(-1, 128).T)
        val_tbl = np.ascontiguousarray(val_st.reshape(-1, 128).T)
        out.append({"idx": idx_tbl, "denc": denc_tbl, "val": val_tbl})
    return out


def prep_host(H_l, edge_row, edge_col, edge_val, out_degree, in_degree,
              hop_attention, theta_out, theta_in, Theta,
              n_nodes=N_NODES, shard=SHARD, groups=GROUPS, gpb=GPB,
              chunk=CHUNK, min_cap=MIN_CAP):
    """Host-side preprocessing: per-core input maps + meta for the builder."""
    nodes_pad = shard * NCORES
    H = np.asarray(H_l, dtype=np.float32)
    er = np.asarray(edge_row, dtype=np.int64)
    ec = np.asarray(edge_col, dtype=np.int64)
    ev = np.asarray(edge_val, dtype=np.float32)
    od = np.asarray(out_degree, dtype=np.float32)
    idg = np.asarray(in_degree, dtype=np.float32)

    alpha = _softmax(np.asarray(hop_attention, dtype=np.float64))
    th_o = np.asarray(theta_out, dtype=np.float64)
    th_i = np.asarray(theta_in, dtype=np.float64)
    coef = [(float(alpha[k] * th_o[k]), float(alpha[k] * th_i[k]))
            for k in range(len(alpha))]

    cap = min_cap
    while True:
        try:
            # dir 0 ("out" chain): dest=row, src=col; dir 1: transposed
            t0 = _prep_direction(er, ec, ev, shard, groups, gpb, cap, chunk)
            t1 = _prep_direction(ec, er, ev, shard, groups, gpb, cap, chunk)
            break
        except OverflowError as e:
            cap = max(cap + 1, int(e.args[0]))

    def _tf32(x):
        i = np.asarray(x, np.float32).view(np.int32)
        return ((i + 0x1000) & ~0x1FFF).astype(np.int32).view(np.float32)

    x0o = np.zeros((nodes_pad, D), dtype=np.float32)
    x0i = np.zeros((nodes_pad, D), dtype=np.float32)
    x0o[:n_nodes] = _tf32(np.maximum(od, 1e-8)[:, None] * H)
    x0i[:n_nodes] = _tf32(np.maximum(idg, 1e-8)[:, None] * H)

    hpad = np.zeros((nodes_pad, D), dtype=np.float32)
    hpad[:n_nodes] = H
    ident = np.eye(128, dtype=np.float32)
    theta = np.ascontiguousarray(np.asarray(Theta, dtype=np.float32))

    in_maps = []
    for m in range(NCORES):
        in_maps.append({
            "x0_out": x0o,
            "x0_in": x0i,
            "hfm": np.ascontiguousarray(hpad[m * shard:(m + 1) * shard].T),
            "theta": theta,
            "ident": ident,
            "idx0": t0[m]["idx"], "denc0": t0[m]["denc"], "val0": t0[m]["val"],
            "idx1": t1[m]["idx"], "denc1": t1[m]["denc"], "val1": t1[m]["val"],
        })
    meta = {"cap": cap, "coef": coef, "shard": shard, "groups": groups,
            "gpb": gpb, "chunk": chunk, "nodes_pad": nodes_pad}
    return in_maps, meta


def build_program(tc, ins, outs, meta):
    """Emit the full SPMD program into TileContext tc.

    ins/outs: dicts of bass APs (DRAM), keys as in prep_host in_maps + "y".
    """
    import concourse.mybir as mybir

    nc = tc.nc
    f32 = mybir.dt.float32
    f32r = mybir.dt.float32r
    i16 = mybir.dt.int16
    EQ, MUL, ADD = (mybir.AluOpType.is_equal, mybir.AluOpType.mult,
                    mybir.AluOpType.add)

    cap = meta["cap"]
    coef = meta["coef"]
    shard = meta["shard"]
    groups = meta["groups"]
    gpb = meta["gpb"]
    chunk = meta["chunk"]
    nodes_pad = meta["nodes_pad"]
    cap_e = cap * 128
    blocks, block_base, tot = _layout(groups, gpb, cap)
    nslots_tot = tot // 128
    rg = [list(range(NCORES))]

    # internal DRAM: per-direction bounce shard + ping-pong full buffers
    bounce = [nc.dram_tensor(f"bounce{d}", [shard, D], f32r,
                            kind="Internal") for d in range(2)]
    xbuf = [[nc.dram_tensor(f"xbuf{d}_{p}", [nodes_pad, D], f32r,
                            kind="Internal", addr_space="Shared")
             for p in range(2)] for d in range(2)]

    tabs = [
        (ins["idx0"], ins["denc0"], ins["val0"]),
        (ins["idx1"], ins["denc1"], ins["val1"]),
    ]
    x0 = [ins["x0_out"], ins["x0_in"]]

    with (
        tc.tile_pool(name="const", bufs=1) as cpool,
        tc.tile_pool(name="work", bufs=1) as wpool,
        tc.tile_pool(name="stream", bufs=4) as spool,
        tc.tile_pool(name="spool2", bufs=4) as spool2,
        tc.tile_pool(name="fin", bufs=2) as fpool,
        tc.tile_pool(name="ps", bufs=4, space="PSUM") as pspool,
        tc.tile_pool(name="psf", bufs=2, space="PSUM") as psfpool,
    ):
        iota = cpool.tile([128, gpb * cap * 128], f32, tag="iota")
        nc.gpsimd.iota(iota[:], pattern=[[0, gpb * cap], [1, 128]], base=0,
                       channel_multiplier=0,
                       allow_small_or_imprecise_dtypes=True)
        ident_s = cpool.tile([128, 128], f32, tag="ident")
        nc.sync.dma_start(ident_s[:], ins["ident"][:])
        theta_s = cpool.tile([64, D], f32, tag="theta")
        nc.sync.dma_start(theta_s[:], ins["theta"][:])

        st = wpool.tile([128, groups, D], f32, tag="st")
        nc.vector.memset(st[:], 0.0)

        for hop in range(0 if SKIP_SPMM else NUM_HOPS):
            for dirn in range(2):
                idx_d, denc_d, val_d = tabs[dirn]
                xsrc = x0[dirn] if hop == 0 else xbuf[dirn][(hop - 1) % 2]

                denc_s = wpool.tile([128, nslots_tot], f32,
                                    tag=f"denc{dirn}")
                val_s = wpool.tile([128, nslots_tot], f32, tag=f"val{dirn}")
                nc.sync.dma_start(denc_s[:], denc_d[:])
                nc.sync.dma_start(val_s[:], val_d[:])

                xnew = wpool.tile([128, groups, D], f32r, tag=f"xnew{dirn}")

                for bi, (g0, gc) in enumerate(blocks):
                    L = gc * cap_e           # idxs per call
                    ns = gc * cap            # slots per call
                    per_call = []
                    for c in range(NCHUNKS):
                        eoff = block_base[bi] + c * L
                        soff = eoff // 128
                        idx_t = spool.tile([128, L // 16], i16, tag="idx")
                        nc.sync.dma_start(
                            idx_t[:], idx_d[:, eoff // 16:(eoff + L) // 16])
                        msgs = spool.tile([128, ns, D], f32r, tag="msgs")
                        nc.gpsimd.dma_gather(
                            out_ap=msgs[:],
                            in_ap=xsrc[c * chunk:(c + 1) * chunk, :].bitcast(
                                f32r),
                            idxs_ap=idx_t[:],
                            num_idxs=L,
                            num_idxs_reg=L,
                            elem_size=D,
                            single_packet=False,
                            queue_num=c,
                        )
                        S = spool2.tile([128, ns, 128], f32r, tag="S")
                        iota_v = iota[:].rearrange(
                            "p (s c) -> p s c", c=128)[:, :ns, :]
                        nc.vector.tensor_tensor(
                            out=S[:], in0=iota_v,
                            in1=denc_s[:, soff:soff + ns].broadcast_to(
                                [128, ns, 128]),
                            op=EQ)
                        nc.vector.tensor_tensor(
                            out=S[:], in0=S[:].bitcast(f32),
                            in1=val_s[:, soff:soff + ns].broadcast_to(
                                [128, ns, 128]),
                            op=MUL)
                        per_call.append((msgs, S))

                    for gl in range(gc):
                        g = g0 + gl
                        ps = pspool.tile([128, D], f32, tag="ps")
                        for c in range(NCHUNKS):
                            msgs, S = per_call[c]
                            for s in range(cap):
                                sl = gl * cap + s
                                nc.tensor.matmul(
                                    ps[:],
                                    lhsT=S[:, sl, :],
                                    rhs=msgs[:, sl, :],
                                    start=(c == 0 and s == 0),
                                    stop=(c == NCHUNKS - 1 and s == cap - 1),
                                )
                        nc.scalar.copy(out=xnew[:, g, :], in_=ps[:])
                        nc.vector.scalar_tensor_tensor(
                            out=st[:, g, :], in0=ps[:],
                            scalar=coef[hop][dirn], in1=st[:, g, :],
                            op0=MUL, op1=ADD)

                bounce_v = bounce[dirn].ap().rearrange(
                    "(g p) f -> p g f", p=128)
                nc.sync.dma_start(bounce_v, xnew[:])
                if hop < NUM_HOPS - 1:
                    nc.gpsimd.collective_compute(
                        "AllGather", mybir.AluOpType.bypass,
                        replica_groups=rg,
                        ins=[bounce[dirn].ap().opt()],
                        outs=[xbuf[dirn][hop % 2].ap().opt()],
                    )

        # final: y_fm = sigmoid(Theta^T @ st_fm) + H_fm, feature-major
        if SKIP_FINAL:
            for g in range(groups):
                yv = outs["y"][:, g * 128:(g + 1) * 128].rearrange(
                    "f p -> p f")
                nc.sync.dma_start(yv, st[:, g, :])
            return
        fchunks = [(i * 4, min(4, groups - i * 4))
                   for i in range((groups + 3) // 4)]
        for ci, (gs, gcnt) in enumerate(fchunks):
            width = gcnt * 128
            stfm = fpool.tile([64, width], f32, tag="stfm")
            for j in range(gcnt):
                pt = psfpool.tile([64, 128], f32, tag="pt")
                nc.tensor.transpose(pt[:], st[:, gs + j, :], ident_s[:])
                nc.scalar.copy(out=stfm[:, j * 128:(j + 1) * 128], in_=pt[:])
            zp = psfpool.tile([64, width], f32, tag="zp")
            nc.tensor.matmul(zp[:], lhsT=theta_s[:], rhs=stfm[:],
                             start=True, stop=True)
            sg = fpool.tile([64, width], f32, tag="sg")
            nc.scalar.activation(sg[:], zp[:],
                                 mybir.ActivationFunctionType.Sigmoid)
            hf = fpool.tile([64, width], f32, tag="hf")
            nc.sync.dma_start(
                hf[:], ins["hfm"][:, gs * 128:gs * 128 + width])
            yt = fpool.tile([64, width], f32, tag="yt")
            nc.vector.tensor_tensor(out=yt[:], in0=sg[:], in1=hf[:], op=ADD)
            nc.sync.dma_start(
                outs["y"][:, gs * 128:gs * 128 + width], yt[:])


def kernel(**inputs) -> np.ndarray:
    return _run(inputs, trace=False)[0]


def kernel_traced(inputs, trace_kwargs=None):
    """Returns (output, BassKernelResults) with NTFF trace if available."""
    return _run(inputs, trace=True, trace_kwargs=trace_kwargs or {})


def _run(inputs, trace=False, trace_kwargs=None):
    import concourse.bacc as bacc
    import concourse.mybir as mybir
    import concourse.tile as tile
    from concourse.bass_utils import run_bass_kernel_spmd

    in_maps, meta = prep_host(**inputs)

    nc = bacc.Bacc("TRN2", target_bir_lowering=False, debug=False,
                   num_devices=NCORES, num_swdge_queues=4)
    f32 = mybir.dt.float32
    f32r = mybir.dt.float32r
    i16 = mybir.dt.int16
    tot = _layout(meta["groups"], meta["gpb"], meta["cap"])[2]

    ins = {}
    shapes = {
        "x0_out": ([meta["nodes_pad"], D], f32r),
        "x0_in": ([meta["nodes_pad"], D], f32r),
        "hfm": ([D, meta["shard"]], f32),
        "theta": ([D, D], f32),
        "ident": ([128, 128], f32),
        "idx0": ([128, tot // 16], i16),
        "denc0": ([128, tot // 128], f32),
        "val0": ([128, tot // 128], f32),
        "idx1": ([128, tot // 16], i16),
        "denc1": ([128, tot // 128], f32),
        "val1": ([128, tot // 128], f32),
    }
    for k, (shape, dt) in shapes.items():
        ins[k] = nc.dram_tensor(k, shape, dt, kind="ExternalInput").ap()
    y = nc.dram_tensor("y", [D, meta["shard"]], f32, kind="ExternalOutput")

    with tile.TileContext(nc) as tc:
        build_program(tc, ins, {"y": y.ap()}, meta)
    nc.compile()

    kw = {}
    if trace:
        kw = dict(trace=True, trace_kwargs=trace_kwargs or {})
    res = run_bass_kernel_spmd(nc, in_maps, core_ids=list(range(NCORES)),
                               **kw)
    shards = [r["y"].T for r in res.results]  # each [shard, 64]
    out = np.concatenate(shards, axis=0)[:N_NODES]
    return np.ascontiguousarray(out.astype(np.float32)), res



# revision 8
# speedup vs baseline: 1.4054x; 1.4054x over previous
"""CascadeGDCN (3-hop graph diffusion convolution) on 8 Trainium2 NeuronCores.

Strategy (matches the sharding hint):
  - Destination nodes sharded across the 8 cores (12544 rows each, padded to
    100352 total).  Edges partitioned by destination core.
  - The full feature matrix X is replicated in every core's DRAM in fp16 with
    rows padded to 128 elements (256B, the dma_gather stride granularity);
    after each hop an AllGather on the tight [shard, 64] fp16 shards rebuilds
    a tight full copy which a strided DMA expands into the padded layout.
  - Per SpMM: per-core edges are bucketed into cells (128-dest "group" x
    25088-row source "chunk"; the chunk split keeps dma_gather's int16 source
    indices in range).  Messages are fetched with dma_gather (256B rows,
    4 SWDGE queues round-robin so the drains overlap), the segment reduction
    runs on the TensorEngine as S^T @ M where S is a [128 edges x 128 dests]
    one-hot-times-val fp16 matrix built on-chip by two batched VectorEngine
    ops (iota compare + multiply).  PSUM (f32) accumulates the 20 tiles
    (4 chunks x 5 slots) of each dest group.
  - Final stage: sum_term (kept f32) transposed via PE, z = Theta^T @ st_fm,
    sigmoid on ScalarE, + H on VectorE, output written feature-major and
    transposed back on the host.
"""

import numpy as np

D = 64
DPAD = 128               # fp16 row padded to 256B for the gather stride
NCORES = 8
NUM_HOPS = 3
N_NODES = 100000
SHARD = 12544            # dest rows per core (98 groups of 128)
NODES_PAD = SHARD * NCORES   # 100352
NCHUNKS = 4
CHUNK = NODES_PAD // NCHUNKS  # 25088 (< 32768 so chunk-local idx fits int16)
GROUPS = SHARD // 128    # 98
GPB = 4                  # dest groups per block (per gather call)
MIN_CAP = 5              # min slots (128-edge tiles) per (group, chunk) cell
SKIP_FINAL = False       # dev bisect flag
SKIP_SPMM = False        # dev bisect flag


def _softmax(x):
    e = np.exp(x - x.max())
    return e / e.sum()


def _blocks(groups, gpb):
    out = []
    g = 0
    while g < groups:
        out.append((g, min(gpb, groups - g)))
        g += gpb
    return out


def _layout(groups, gpb, cap):
    """Edge-stream layout: [block][chunk][group_in_block][cap*128]."""
    cap_e = cap * 128
    blocks = _blocks(groups, gpb)
    block_base = []
    base = 0
    for _, gc in blocks:
        block_base.append(base)
        base += NCHUNKS * gc * cap_e
    return blocks, block_base, base  # base == total edge slots


def _prep_direction(dest, src, val, shard, groups, gpb, cap, chunk):
    """Build per-core gather/S tables for one SpMM direction.

    Returns list (per core) of dicts with idx/denc/val device tables.
    """
    cap_e = cap * 128
    blocks, block_base, tot = _layout(groups, gpb, cap)
    ncells = groups * NCHUNKS

    # cell base offset for (g, c)
    cell_base = np.empty(ncells, dtype=np.int64)
    for bi, (g0, gc) in enumerate(blocks):
        for gl in range(gc):
            for c in range(NCHUNKS):
                cell_base[(g0 + gl) * NCHUNKS + c] = (
                    block_base[bi] + c * gc * cap_e + gl * cap_e
                )

    core = dest // shard
    out = []
    for m in range(NCORES):
        sel = core == m
        d_loc = (dest[sel] - m * shard).astype(np.int64)
        s = src[sel].astype(np.int64)
        v = val[sel].astype(np.float32)
        g = d_loc >> 7
        c = s // chunk
        cell = g * NCHUNKS + c
        order = np.argsort(cell, kind="stable")
        cell_s = cell[order]
        counts = np.bincount(cell_s, minlength=ncells)
        if counts.max() > cap_e:
            raise OverflowError(int(np.ceil(counts.max() / 128)))
        starts = np.zeros(ncells, dtype=np.int64)
        starts[1:] = np.cumsum(counts)[:-1]
        rank = np.arange(cell_s.size) - starts[cell_s]
        pos = cell_base[cell_s] + rank

        idx_st = np.zeros(tot, dtype=np.int16)
        denc_st = np.full(tot, -1.0, dtype=np.float16)
        val_st = np.zeros(tot, dtype=np.float16)
        idx_st[pos] = (s[order] - c[order] * chunk).astype(np.int16)
        denc_st[pos] = (d_loc[order] & 127).astype(np.float16)
        val_st[pos] = v[order].astype(np.float16)

        # the 16-row wrapped block must be replicated into all 8 Q7-core
        # stripes (hardware reads its own 16-partition stripe)
        idx_tbl = np.tile(np.ascontiguousarray(idx_st.reshape(-1, 16).T),
                          (8, 1))
        denc_tbl = np.ascontiguousarray(denc_st.reshape(-1, 128).T)
        val_tbl = np.ascontiguousarray(val_st.reshape(-1, 128).T)
        out.append({"idx": idx_tbl, "denc": denc_tbl, "val": val_tbl})
    return out


def prep_host(H_l, edge_row, edge_col, edge_val, out_degree, in_degree,
              hop_attention, theta_out, theta_in, Theta,
              n_nodes=N_NODES, shard=SHARD, groups=GROUPS, gpb=GPB,
              chunk=CHUNK, min_cap=MIN_CAP):
    """Host-side preprocessing: per-core input maps + meta for the builder."""
    nodes_pad = shard * NCORES
    H = np.asarray(H_l, dtype=np.float32)
    er = np.asarray(edge_row, dtype=np.int64)
    ec = np.asarray(edge_col, dtype=np.int64)
    ev = np.asarray(edge_val, dtype=np.float32)
    od = np.asarray(out_degree, dtype=np.float32)
    idg = np.asarray(in_degree, dtype=np.float32)

    alpha = _softmax(np.asarray(hop_attention, dtype=np.float64))
    th_o = np.asarray(theta_out, dtype=np.float64)
    th_i = np.asarray(theta_in, dtype=np.float64)
    coef = [(float(alpha[k] * th_o[k]), float(alpha[k] * th_i[k]))
            for k in range(len(alpha))]

    cap = min_cap
    while True:
        try:
            # dir 0 ("out" chain): dest=row, src=col; dir 1: transposed
            t0 = _prep_direction(er, ec, ev, shard, groups, gpb, cap, chunk)
            t1 = _prep_direction(ec, er, ev, shard, groups, gpb, cap, chunk)
            break
        except OverflowError as e:
            cap = max(cap + 1, int(e.args[0]))

    x0o = np.zeros((nodes_pad, DPAD), dtype=np.float16)
    x0i = np.zeros((nodes_pad, DPAD), dtype=np.float16)
    x0o[:n_nodes, :D] = (np.maximum(od, 1e-8)[:, None] * H).astype(np.float16)
    x0i[:n_nodes, :D] = (np.maximum(idg, 1e-8)[:, None] * H).astype(np.float16)

    hpad = np.zeros((nodes_pad, D), dtype=np.float32)
    hpad[:n_nodes] = H
    ident = np.eye(128, dtype=np.float32)
    theta = np.ascontiguousarray(np.asarray(Theta, dtype=np.float32))

    in_maps = []
    for m in range(NCORES):
        in_maps.append({
            "x0_out": x0o,
            "x0_in": x0i,
            "hfm": np.ascontiguousarray(hpad[m * shard:(m + 1) * shard].T),
            "theta": theta,
            "ident": ident,
            "idx0": t0[m]["idx"], "denc0": t0[m]["denc"], "val0": t0[m]["val"],
            "idx1": t1[m]["idx"], "denc1": t1[m]["denc"], "val1": t1[m]["val"],
        })
    meta = {"cap": cap, "coef": coef, "shard": shard, "groups": groups,
            "gpb": gpb, "chunk": chunk, "nodes_pad": nodes_pad}
    return in_maps, meta


def build_program(tc, ins, outs, meta):
    """Emit the full SPMD program into TileContext tc.

    ins/outs: dicts of bass APs (DRAM), keys as in prep_host in_maps + "y".
    """
    import concourse.mybir as mybir

    nc = tc.nc
    f32 = mybir.dt.float32
    f16 = mybir.dt.float16
    i16 = mybir.dt.int16
    EQ, MUL, ADD = (mybir.AluOpType.is_equal, mybir.AluOpType.mult,
                    mybir.AluOpType.add)

    cap = meta["cap"]
    coef = meta["coef"]
    shard = meta["shard"]
    groups = meta["groups"]
    gpb = meta["gpb"]
    chunk = meta["chunk"]
    nodes_pad = meta["nodes_pad"]
    cap_e = cap * 128
    blocks, block_base, tot = _layout(groups, gpb, cap)
    nslots_tot = tot // 128
    rg = [list(range(NCORES))]

    # internal DRAM: per-direction bounce shard + tight AG output (ping-pong)
    # + padded gather source (ping-pong)
    bounce = [[nc.dram_tensor(f"bounce{d}_{p}", [shard, D], f16,
                              kind="Internal") for p in range(2)]
              for d in range(2)]
    xtight = [[nc.dram_tensor(f"xtight{d}_{p}", [nodes_pad, D], f16,
                              kind="Internal", addr_space="Shared")
               for p in range(2)] for d in range(2)]
    xpad = [[nc.dram_tensor(f"xpad{d}_{p}", [nodes_pad, DPAD], f16,
                            kind="Internal") for p in range(2)]
            for d in range(2)]

    tabs = [
        (ins["idx0"], ins["denc0"], ins["val0"]),
        (ins["idx1"], ins["denc1"], ins["val1"]),
    ]
    x0 = [ins["x0_out"], ins["x0_in"]]

    with (
        tc.tile_pool(name="const", bufs=1) as cpool,
        tc.tile_pool(name="work", bufs=1) as wpool,
        tc.tile_pool(name="stream", bufs=8) as spool,
        tc.tile_pool(name="spool2", bufs=6) as spool2,
        tc.tile_pool(name="fin", bufs=2) as fpool,
        tc.tile_pool(name="ps", bufs=4, space="PSUM") as pspool,
        tc.tile_pool(name="psf", bufs=2, space="PSUM") as psfpool,
    ):
        iota = cpool.tile([128, gpb * cap * 128], f16, tag="iota")
        nc.gpsimd.iota(iota[:], pattern=[[0, gpb * cap], [1, 128]], base=0,
                       channel_multiplier=0,
                       allow_small_or_imprecise_dtypes=True)
        ident_s = cpool.tile([128, 128], f32, tag="ident")
        nc.sync.dma_start(ident_s[:], ins["ident"][:])
        theta_s = cpool.tile([64, D], f32, tag="theta")
        nc.sync.dma_start(theta_s[:], ins["theta"][:])

        # denc/val tables: load once per direction (reused across hops)
        denc_s = []
        val_s = []
        for dirn in range(2):
            dt_ = cpool.tile([128, nslots_tot], f16, tag=f"denc{dirn}")
            vt_ = cpool.tile([128, nslots_tot], f16, tag=f"val{dirn}")
            nc.sync.dma_start(dt_[:], tabs[dirn][1][:])
            nc.sync.dma_start(vt_[:], tabs[dirn][2][:])
            denc_s.append(dt_)
            val_s.append(vt_)

        st = wpool.tile([128, groups, D], f32, tag="st")
        nc.vector.memset(st[:], 0.0)
        xnew0 = wpool.tile([128, groups, D], f16, tag="xnew0")
        xnew1 = wpool.tile([128, groups, D], f16, tag="xnew1")
        xnew = [xnew0, xnew1]

        for hop in range(0 if SKIP_SPMM else NUM_HOPS):
            for dirn in range(2):
                idx_d = tabs[dirn][0]
                xsrc = (x0[dirn] if hop == 0
                        else xpad[dirn][(hop - 1) % 2].ap())

                for bi, (g0, gc) in enumerate(blocks):
                    L = gc * cap_e           # idxs per call
                    ns = gc * cap            # slots per call
                    per_call = []
                    for c in range(NCHUNKS):
                        eoff = block_base[bi] + c * L
                        soff = eoff // 128
                        idx_t = spool.tile([128, L // 16], i16, tag="idx")
                        nc.sync.dma_start(
                            idx_t[:], idx_d[:, eoff // 16:(eoff + L) // 16])
                        msgs = spool.tile([128, ns, DPAD], f16, tag="msgs")
                        nc.gpsimd.dma_gather(
                            out_ap=msgs[:],
                            in_ap=xsrc[c * chunk:(c + 1) * chunk, :],
                            idxs_ap=idx_t[:],
                            num_idxs=L,
                            num_idxs_reg=L,
                            elem_size=DPAD,
                            single_packet=False,
                            queue_num=c,
                        )
                        S = spool2.tile([128, ns, 128], f16, tag="S")
                        iota_v = iota[:].rearrange(
                            "p (s c) -> p s c", c=128)[:, :ns, :]
                        nc.vector.tensor_tensor(
                            out=S[:], in0=iota_v,
                            in1=denc_s[dirn][:, soff:soff + ns].broadcast_to(
                                [128, ns, 128]),
                            op=EQ)
                        nc.vector.tensor_tensor(
                            out=S[:], in0=S[:],
                            in1=val_s[dirn][:, soff:soff + ns].broadcast_to(
                                [128, ns, 128]),
                            op=MUL)
                        per_call.append((msgs, S))

                    for gl in range(gc):
                        g = g0 + gl
                        ps = pspool.tile([128, D], f32, tag="ps")
                        for c in range(NCHUNKS):
                            msgs, S = per_call[c]
                            for s in range(cap):
                                sl = gl * cap + s
                                nc.tensor.matmul(
                                    ps[:],
                                    lhsT=S[:, sl, :],
                                    rhs=msgs[:, sl, 0:D],
                                    start=(c == 0 and s == 0),
                                    stop=(c == NCHUNKS - 1 and s == cap - 1),
                                )
                        nc.scalar.copy(out=xnew[dirn][:, g, :], in_=ps[:])
                        nc.vector.scalar_tensor_tensor(
                            out=st[:, g, :], in0=ps[:],
                            scalar=coef[hop][dirn], in1=st[:, g, :],
                            op0=MUL, op1=ADD)

                bounce_v = bounce[dirn][hop % 2].ap().rearrange(
                    "(g p) f -> p g f", p=128)
                nc.sync.dma_start(bounce_v, xnew[dirn][:])
                if hop < NUM_HOPS - 1:
                    nc.gpsimd.collective_compute(
                        "AllGather", mybir.AluOpType.bypass,
                        replica_groups=rg,
                        ins=[bounce[dirn][hop % 2].ap().opt()],
                        outs=[xtight[dirn][hop % 2].ap().opt()],
                    )
                    # expand tight [nodes,64] into padded [nodes,128] rows
                    # (split: DMA AP dims are 16-bit, 100352 > 65535)
                    half = nodes_pad // 2
                    for hh in range(2):
                        nc.sync.dma_start(
                            xpad[dirn][hop % 2].ap()[hh * half:(hh + 1) * half,
                                                     0:D],
                            xtight[dirn][hop % 2].ap()[hh * half:(hh + 1) * half,
                                                       :])

        # final: y_fm = sigmoid(Theta^T @ st_fm) + H_fm, feature-major
        if SKIP_FINAL:
            for g in range(groups):
                yv = outs["y"][:, g * 128:(g + 1) * 128].rearrange(
                    "f p -> p f")
                nc.sync.dma_start(yv, st[:, g, :])
            return
        fchunks = [(i * 4, min(4, groups - i * 4))
                   for i in range((groups + 3) // 4)]
        for ci, (gs, gcnt) in enumerate(fchunks):
            width = gcnt * 128
            stfm = fpool.tile([64, width], f32, tag="stfm")
            for j in range(gcnt):
                pt = psfpool.tile([64, 128], f32, tag="pt")
                nc.tensor.transpose(pt[:], st[:, gs + j, :], ident_s[:])
                nc.scalar.copy(out=stfm[:, j * 128:(j + 1) * 128], in_=pt[:])
            zp = psfpool.tile([64, width], f32, tag="zp")
            nc.tensor.matmul(zp[:], lhsT=theta_s[:], rhs=stfm[:],
                             start=True, stop=True)
            sg = fpool.tile([64, width], f32, tag="sg")
            nc.scalar.activation(sg[:], zp[:],
                                 mybir.ActivationFunctionType.Sigmoid)
            hf = fpool.tile([64, width], f32, tag="hf")
            nc.sync.dma_start(
                hf[:], ins["hfm"][:, gs * 128:gs * 128 + width])
            yt = fpool.tile([64, width], f32, tag="yt")
            nc.vector.tensor_tensor(out=yt[:], in0=sg[:], in1=hf[:], op=ADD)
            nc.sync.dma_start(
                outs["y"][:, gs * 128:gs * 128 + width], yt[:])


def kernel(**inputs) -> np.ndarray:
    return _run(inputs, trace=False)[0]


def kernel_traced(inputs, trace_kwargs=None):
    """Returns (output, BassKernelResults) with NTFF trace if available."""
    return _run(inputs, trace=True, trace_kwargs=trace_kwargs or {})


def _run(inputs, trace=False, trace_kwargs=None):
    import concourse.bacc as bacc
    import concourse.mybir as mybir
    import concourse.tile as tile
    from concourse.bass_utils import run_bass_kernel_spmd

    in_maps, meta = prep_host(**inputs)

    nc = bacc.Bacc("TRN2", target_bir_lowering=False, debug=False,
                   num_devices=NCORES, num_swdge_queues=4)
    f32 = mybir.dt.float32
    f16 = mybir.dt.float16
    i16 = mybir.dt.int16
    tot = _layout(meta["groups"], meta["gpb"], meta["cap"])[2]

    ins = {}
    shapes = {
        "x0_out": ([meta["nodes_pad"], DPAD], f16),
        "x0_in": ([meta["nodes_pad"], DPAD], f16),
        "hfm": ([D, meta["shard"]], f32),
        "theta": ([D, D], f32),
        "ident": ([128, 128], f32),
        "idx0": ([128, tot // 16], i16),
        "denc0": ([128, tot // 128], f16),
        "val0": ([128, tot // 128], f16),
        "idx1": ([128, tot // 16], i16),
        "denc1": ([128, tot // 128], f16),
        "val1": ([128, tot // 128], f16),
    }
    for k, (shape, dt) in shapes.items():
        ins[k] = nc.dram_tensor(k, shape, dt, kind="ExternalInput").ap()
    y = nc.dram_tensor("y", [D, meta["shard"]], f32, kind="ExternalOutput")

    with tile.TileContext(nc) as tc:
        build_program(tc, ins, {"y": y.ap()}, meta)
    nc.compile()

    kw = {}
    if trace:
        kw = dict(trace=True, trace_kwargs=trace_kwargs or {})
    res = run_bass_kernel_spmd(nc, in_maps, core_ids=list(range(NCORES)),
                               **kw)
    shards = [r["y"].T for r in res.results]  # each [shard, 64]
    out = np.concatenate(shards, axis=0)[:N_NODES]
    return np.ascontiguousarray(out.astype(np.float32)), res


# revision 9
# speedup vs baseline: 1.4746x; 1.0492x over previous
"""CascadeGDCN (3-hop graph diffusion convolution) on 8 Trainium2 NeuronCores.

Strategy (matches the sharding hint):
  - Destination nodes sharded across the 8 cores (12544 rows each, padded to
    100352 total).  Edges partitioned by destination core.
  - The full feature matrix X is replicated in every core's DRAM in fp16 with
    rows padded to 128 elements (256B, the dma_gather stride granularity);
    after each hop an AllGather on the tight [shard, 64] fp16 shards rebuilds
    a tight full copy which a strided DMA expands into the padded layout.
  - Per SpMM: per-core edges are bucketed into cells (128-dest "group" x
    25088-row source "chunk"; the chunk split keeps dma_gather's int16 source
    indices in range).  Messages are fetched with dma_gather (256B rows,
    4 SWDGE queues round-robin so the drains overlap), the segment reduction
    runs on the TensorEngine as S^T @ M where S is a [128 edges x 128 dests]
    one-hot-times-val fp16 matrix built on-chip by two batched VectorEngine
    ops (iota compare + multiply).  PSUM (f32) accumulates the 20 tiles
    (4 chunks x 5 slots) of each dest group.
  - Final stage: sum_term (kept f32) transposed via PE, z = Theta^T @ st_fm,
    sigmoid on ScalarE, + H on VectorE, output written feature-major and
    transposed back on the host.
"""

import numpy as np

D = 64
DPAD = 128               # fp16 row padded to 256B for the gather stride
NCORES = 8
NUM_HOPS = 3
N_NODES = 100000
SHARD = 12544            # dest rows per core (98 groups of 128)
NODES_PAD = SHARD * NCORES   # 100352
NCHUNKS = 4
CHUNK = NODES_PAD // NCHUNKS  # 25088 (< 32768 so chunk-local idx fits int16)
GROUPS = SHARD // 128    # 98
GPB = 4                  # dest groups per block (per gather call)
MIN_CAP = 5              # min slots (128-edge tiles) per (group, chunk) cell
SKIP_FINAL = False       # dev bisect flag
SKIP_SPMM = False        # dev bisect flag


def _softmax(x):
    e = np.exp(x - x.max())
    return e / e.sum()


def _blocks(groups, gpb):
    out = []
    g = 0
    while g < groups:
        out.append((g, min(gpb, groups - g)))
        g += gpb
    return out


def _layout(groups, gpb, cap):
    """Edge-stream layout: [block][chunk][group_in_block][cap*128]."""
    cap_e = cap * 128
    blocks = _blocks(groups, gpb)
    block_base = []
    base = 0
    for _, gc in blocks:
        block_base.append(base)
        base += NCHUNKS * gc * cap_e
    return blocks, block_base, base  # base == total edge slots


def _prep_direction(dest, src, val, shard, groups, gpb, cap, chunk):
    """Build per-core gather/S tables for one SpMM direction.

    Returns list (per core) of dicts with idx/denc/val device tables.
    """
    cap_e = cap * 128
    blocks, block_base, tot = _layout(groups, gpb, cap)
    ncells = groups * NCHUNKS

    # cell base offset for (g, c)
    cell_base = np.empty(ncells, dtype=np.int64)
    for bi, (g0, gc) in enumerate(blocks):
        for gl in range(gc):
            for c in range(NCHUNKS):
                cell_base[(g0 + gl) * NCHUNKS + c] = (
                    block_base[bi] + c * gc * cap_e + gl * cap_e
                )

    core = dest // shard
    out = []
    for m in range(NCORES):
        sel = core == m
        d_loc = (dest[sel] - m * shard).astype(np.int64)
        s = src[sel].astype(np.int64)
        v = val[sel].astype(np.float32)
        g = d_loc >> 7
        c = s // chunk
        cell = g * NCHUNKS + c
        order = np.argsort(cell, kind="stable")
        cell_s = cell[order]
        counts = np.bincount(cell_s, minlength=ncells)
        if counts.max() > cap_e:
            raise OverflowError(int(np.ceil(counts.max() / 128)))
        starts = np.zeros(ncells, dtype=np.int64)
        starts[1:] = np.cumsum(counts)[:-1]
        rank = np.arange(cell_s.size) - starts[cell_s]
        pos = cell_base[cell_s] + rank

        idx_st = np.zeros(tot, dtype=np.int16)
        denc_st = np.full(tot, -1.0, dtype=np.float16)
        val_st = np.zeros(tot, dtype=np.float16)
        idx_st[pos] = (s[order] - c[order] * chunk).astype(np.int16)
        denc_st[pos] = (d_loc[order] & 127).astype(np.float16)
        val_st[pos] = v[order].astype(np.float16)

        # the 16-row wrapped block must be replicated into all 8 Q7-core
        # stripes (hardware reads its own 16-partition stripe)
        idx_tbl = np.tile(np.ascontiguousarray(idx_st.reshape(-1, 16).T),
                          (8, 1))
        denc_tbl = np.ascontiguousarray(denc_st.reshape(-1, 128).T)
        val_tbl = np.ascontiguousarray(val_st.reshape(-1, 128).T)
        out.append({"idx": idx_tbl, "denc": denc_tbl, "val": val_tbl})
    return out


def prep_host(H_l, edge_row, edge_col, edge_val, out_degree, in_degree,
              hop_attention, theta_out, theta_in, Theta,
              n_nodes=N_NODES, shard=SHARD, groups=GROUPS, gpb=GPB,
              chunk=CHUNK, min_cap=MIN_CAP):
    """Host-side preprocessing: per-core input maps + meta for the builder."""
    nodes_pad = shard * NCORES
    H = np.asarray(H_l, dtype=np.float32)
    er = np.asarray(edge_row, dtype=np.int64)
    ec = np.asarray(edge_col, dtype=np.int64)
    ev = np.asarray(edge_val, dtype=np.float32)
    od = np.asarray(out_degree, dtype=np.float32)
    idg = np.asarray(in_degree, dtype=np.float32)

    alpha = _softmax(np.asarray(hop_attention, dtype=np.float64))
    th_o = np.asarray(theta_out, dtype=np.float64)
    th_i = np.asarray(theta_in, dtype=np.float64)
    coef = [(float(alpha[k] * th_o[k]), float(alpha[k] * th_i[k]))
            for k in range(len(alpha))]

    cap = min_cap
    while True:
        try:
            # dir 0 ("out" chain): dest=row, src=col; dir 1: transposed
            t0 = _prep_direction(er, ec, ev, shard, groups, gpb, cap, chunk)
            t1 = _prep_direction(ec, er, ev, shard, groups, gpb, cap, chunk)
            break
        except OverflowError as e:
            cap = max(cap + 1, int(e.args[0]))

    x0o = np.zeros((nodes_pad, DPAD), dtype=np.float16)
    x0i = np.zeros((nodes_pad, DPAD), dtype=np.float16)
    x0o[:n_nodes, :D] = (np.maximum(od, 1e-8)[:, None] * H).astype(np.float16)
    x0i[:n_nodes, :D] = (np.maximum(idg, 1e-8)[:, None] * H).astype(np.float16)

    hpad = np.zeros((nodes_pad, D), dtype=np.float32)
    hpad[:n_nodes] = H
    ident = np.eye(128, dtype=np.float32)
    theta = np.ascontiguousarray(np.asarray(Theta, dtype=np.float32))

    in_maps = []
    for m in range(NCORES):
        in_maps.append({
            "x0_out": x0o,
            "x0_in": x0i,
            "hfm": np.ascontiguousarray(hpad[m * shard:(m + 1) * shard].T),
            "theta": theta,
            "ident": ident,
            "idx0": t0[m]["idx"], "denc0": t0[m]["denc"], "val0": t0[m]["val"],
            "idx1": t1[m]["idx"], "denc1": t1[m]["denc"], "val1": t1[m]["val"],
        })
    meta = {"cap": cap, "coef": coef, "shard": shard, "groups": groups,
            "gpb": gpb, "chunk": chunk, "nodes_pad": nodes_pad}
    return in_maps, meta


def build_program(tc, ins, outs, meta):
    """Emit the full SPMD program into TileContext tc.

    ins/outs: dicts of bass APs (DRAM), keys as in prep_host in_maps + "y".
    """
    import concourse.mybir as mybir

    nc = tc.nc
    f32 = mybir.dt.float32
    f16 = mybir.dt.float16
    i16 = mybir.dt.int16
    EQ, MUL, ADD = (mybir.AluOpType.is_equal, mybir.AluOpType.mult,
                    mybir.AluOpType.add)

    cap = meta["cap"]
    coef = meta["coef"]
    shard = meta["shard"]
    groups = meta["groups"]
    gpb = meta["gpb"]
    chunk = meta["chunk"]
    nodes_pad = meta["nodes_pad"]
    cap_e = cap * 128
    blocks, block_base, tot = _layout(groups, gpb, cap)
    nslots_tot = tot // 128
    rg = [list(range(NCORES))]

    # internal DRAM: per-direction bounce shard + tight AG output (ping-pong)
    # + padded gather source (ping-pong)
    bounce = [[nc.dram_tensor(f"bounce{d}_{p}", [shard, D], f16,
                              kind="Internal") for p in range(2)]
              for d in range(2)]
    xtight = [[nc.dram_tensor(f"xtight{d}_{p}", [nodes_pad, D], f16,
                              kind="Internal", addr_space="Shared")
               for p in range(2)] for d in range(2)]
    xpad = [[nc.dram_tensor(f"xpad{d}_{p}", [nodes_pad, DPAD], f16,
                            kind="Internal") for p in range(2)]
            for d in range(2)]

    tabs = [
        (ins["idx0"], ins["denc0"], ins["val0"]),
        (ins["idx1"], ins["denc1"], ins["val1"]),
    ]
    x0 = [ins["x0_out"], ins["x0_in"]]

    with (
        tc.tile_pool(name="const", bufs=1) as cpool,
        tc.tile_pool(name="work", bufs=1) as wpool,
        tc.tile_pool(name="stream", bufs=8) as spool,
        tc.tile_pool(name="spool2", bufs=6) as spool2,
        tc.tile_pool(name="fin", bufs=2) as fpool,
        tc.tile_pool(name="ps", bufs=4, space="PSUM") as pspool,
        tc.tile_pool(name="psf", bufs=2, space="PSUM") as psfpool,
    ):
        iota = cpool.tile([128, gpb * cap * 128], f16, tag="iota")
        nc.gpsimd.iota(iota[:], pattern=[[0, gpb * cap], [1, 128]], base=0,
                       channel_multiplier=0,
                       allow_small_or_imprecise_dtypes=True)
        ident_s = cpool.tile([128, 128], f32, tag="ident")
        nc.sync.dma_start(ident_s[:], ins["ident"][:])
        theta_s = cpool.tile([64, D], f32, tag="theta")
        nc.sync.dma_start(theta_s[:], ins["theta"][:])

        # denc/val tables: load once per direction (reused across hops)
        denc_s = []
        val_s = []
        for dirn in range(2):
            dt_ = cpool.tile([128, nslots_tot], f16, tag=f"denc{dirn}")
            vt_ = cpool.tile([128, nslots_tot], f16, tag=f"val{dirn}")
            nc.sync.dma_start(dt_[:], tabs[dirn][1][:])
            nc.sync.dma_start(vt_[:], tabs[dirn][2][:])
            denc_s.append(dt_)
            val_s.append(vt_)

        st = wpool.tile([128, groups, D], f32, tag="st")
        nc.vector.memset(st[:], 0.0)
        xnew0 = wpool.tile([128, groups, D], f16, tag="xnew0")
        xnew1 = wpool.tile([128, groups, D], f16, tag="xnew1")
        xnew = [xnew0, xnew1]

        for hop in range(0 if SKIP_SPMM else NUM_HOPS):
            for dirn in range(2):
                idx_d = tabs[dirn][0]
                xsrc = (x0[dirn] if hop == 0
                        else xpad[dirn][(hop - 1) % 2].ap())

                for bi, (g0, gc) in enumerate(blocks):
                    L = gc * cap_e           # idxs per call
                    ns = gc * cap            # slots per call
                    per_call = []
                    for c in range(NCHUNKS):
                        eoff = block_base[bi] + c * L
                        soff = eoff // 128
                        idx_t = spool.tile([128, L // 16], i16, tag="idx")
                        nc.sync.dma_start(
                            idx_t[:], idx_d[:, eoff // 16:(eoff + L) // 16])
                        msgs = spool.tile([128, ns, DPAD], f16, tag="msgs")
                        nc.gpsimd.dma_gather(
                            out_ap=msgs[:],
                            in_ap=xsrc[c * chunk:(c + 1) * chunk, :],
                            idxs_ap=idx_t[:],
                            num_idxs=L,
                            num_idxs_reg=L,
                            elem_size=DPAD,
                            single_packet=False,
                            queue_num=c,
                        )
                        S = spool2.tile([128, ns, 128], f16, tag="S")
                        iota_v = iota[:].rearrange(
                            "p (s c) -> p s c", c=128)[:, :ns, :]
                        nc.vector.tensor_tensor(
                            out=S[:], in0=iota_v,
                            in1=denc_s[dirn][:, soff:soff + ns].broadcast_to(
                                [128, ns, 128]),
                            op=EQ)
                        # scale the 64-wide messages by edge val (half the
                        # elements of scaling the 128-wide S)
                        nc.vector.tensor_tensor(
                            out=msgs[:, :, 0:D], in0=msgs[:, :, 0:D],
                            in1=val_s[dirn][:, soff:soff + ns].broadcast_to(
                                [128, ns, D]),
                            op=MUL)
                        per_call.append((msgs, S))

                    for gl in range(gc):
                        g = g0 + gl
                        ps = pspool.tile([128, D], f32, tag="ps")
                        for c in range(NCHUNKS):
                            msgs, S = per_call[c]
                            for s in range(cap):
                                sl = gl * cap + s
                                nc.tensor.matmul(
                                    ps[:],
                                    lhsT=S[:, sl, :],
                                    rhs=msgs[:, sl, 0:D],
                                    start=(c == 0 and s == 0),
                                    stop=(c == NCHUNKS - 1 and s == cap - 1),
                                )
                        nc.scalar.copy(out=xnew[dirn][:, g, :], in_=ps[:])
                        nc.vector.scalar_tensor_tensor(
                            out=st[:, g, :], in0=ps[:],
                            scalar=coef[hop][dirn], in1=st[:, g, :],
                            op0=MUL, op1=ADD)

                bounce_v = bounce[dirn][hop % 2].ap().rearrange(
                    "(g p) f -> p g f", p=128)
                nc.sync.dma_start(bounce_v, xnew[dirn][:])
                if hop < NUM_HOPS - 1:
                    nc.gpsimd.collective_compute(
                        "AllGather", mybir.AluOpType.bypass,
                        replica_groups=rg,
                        ins=[bounce[dirn][hop % 2].ap().opt()],
                        outs=[xtight[dirn][hop % 2].ap().opt()],
                    )
                    # expand tight [nodes,64] into padded [nodes,128] rows
                    # (split: DMA AP dims are 16-bit, 100352 > 65535)
                    half = nodes_pad // 2
                    for hh in range(2):
                        nc.sync.dma_start(
                            xpad[dirn][hop % 2].ap()[hh * half:(hh + 1) * half,
                                                     0:D],
                            xtight[dirn][hop % 2].ap()[hh * half:(hh + 1) * half,
                                                       :])

        # final: y_fm = sigmoid(Theta^T @ st_fm) + H_fm, feature-major
        if SKIP_FINAL:
            for g in range(groups):
                yv = outs["y"][:, g * 128:(g + 1) * 128].rearrange(
                    "f p -> p f")
                nc.sync.dma_start(yv, st[:, g, :])
            return
        fchunks = [(i * 4, min(4, groups - i * 4))
                   for i in range((groups + 3) // 4)]
        for ci, (gs, gcnt) in enumerate(fchunks):
            width = gcnt * 128
            stfm = fpool.tile([64, width], f32, tag="stfm")
            for j in range(gcnt):
                pt = psfpool.tile([64, 128], f32, tag="pt")
                nc.tensor.transpose(pt[:], st[:, gs + j, :], ident_s[:])
                nc.scalar.copy(out=stfm[:, j * 128:(j + 1) * 128], in_=pt[:])
            zp = psfpool.tile([64, width], f32, tag="zp")
            nc.tensor.matmul(zp[:], lhsT=theta_s[:], rhs=stfm[:],
                             start=True, stop=True)
            sg = fpool.tile([64, width], f32, tag="sg")
            nc.scalar.activation(sg[:], zp[:],
                                 mybir.ActivationFunctionType.Sigmoid)
            hf = fpool.tile([64, width], f32, tag="hf")
            nc.sync.dma_start(
                hf[:], ins["hfm"][:, gs * 128:gs * 128 + width])
            yt = fpool.tile([64, width], f32, tag="yt")
            nc.vector.tensor_tensor(out=yt[:], in0=sg[:], in1=hf[:], op=ADD)
            nc.sync.dma_start(
                outs["y"][:, gs * 128:gs * 128 + width], yt[:])


def kernel(**inputs) -> np.ndarray:
    return _run(inputs, trace=False)[0]


def kernel_traced(inputs, trace_kwargs=None):
    """Returns (output, BassKernelResults) with NTFF trace if available."""
    return _run(inputs, trace=True, trace_kwargs=trace_kwargs or {})


def _run(inputs, trace=False, trace_kwargs=None):
    import concourse.bacc as bacc
    import concourse.mybir as mybir
    import concourse.tile as tile
    from concourse.bass_utils import run_bass_kernel_spmd

    in_maps, meta = prep_host(**inputs)

    nc = bacc.Bacc("TRN2", target_bir_lowering=False, debug=False,
                   num_devices=NCORES, num_swdge_queues=4)
    f32 = mybir.dt.float32
    f16 = mybir.dt.float16
    i16 = mybir.dt.int16
    tot = _layout(meta["groups"], meta["gpb"], meta["cap"])[2]

    ins = {}
    shapes = {
        "x0_out": ([meta["nodes_pad"], DPAD], f16),
        "x0_in": ([meta["nodes_pad"], DPAD], f16),
        "hfm": ([D, meta["shard"]], f32),
        "theta": ([D, D], f32),
        "ident": ([128, 128], f32),
        "idx0": ([128, tot // 16], i16),
        "denc0": ([128, tot // 128], f16),
        "val0": ([128, tot // 128], f16),
        "idx1": ([128, tot // 16], i16),
        "denc1": ([128, tot // 128], f16),
        "val1": ([128, tot // 128], f16),
    }
    for k, (shape, dt) in shapes.items():
        ins[k] = nc.dram_tensor(k, shape, dt, kind="ExternalInput").ap()
    y = nc.dram_tensor("y", [D, meta["shard"]], f32, kind="ExternalOutput")

    with tile.TileContext(nc) as tc:
        build_program(tc, ins, {"y": y.ap()}, meta)
    nc.compile()

    kw = {}
    if trace:
        kw = dict(trace=True, trace_kwargs=trace_kwargs or {})
    res = run_bass_kernel_spmd(nc, in_maps, core_ids=list(range(NCORES)),
                               **kw)
    shards = [r["y"].T for r in res.results]  # each [shard, 64]
    out = np.concatenate(shards, axis=0)[:N_NODES]
    return np.ascontiguousarray(out.astype(np.float32)), res
